# revision 13
# baseline (speedup 1.0000x reference)
"""EdgeScoringNetwork Trainium2 kernel (8 NeuronCores, SPMD).

Sharding: each core handles half a batch (2048 source nodes x 16 edges
= 32768 edges). The generator's edge list is affine — edges are grouped
by source (src = e // DEG) and tgt = (src + d[b,k]) % N with per-(batch,
k) constant offsets — so after the host transposes the l2-normalized
node features to feature-major [D, N] and pre-rolls a per-core copy by
the core's base offset, every 512-edge tile's source/target feature
blocks are contiguous column slices of SBUF-resident tables: no
gathers, no on-device transposes.

Device per 512-edge tile (2-stage software pipeline, tiles t and t-2
in flight): 12 float32r matmuls (1 cycle/row) on PE; exp + the two h1
BN-relus on the scalar engine (one activation table: exp/relu/copy);
a1/h2 relus as fused add+max and the softmax reciprocal on DVE; the
|sf-tf| chain and the cross-partition softmax sum (partition_all_reduce
with built-in broadcast) on GPSIMD. The h2 batch-norm scale is folded
into W3 on the host so h2's relu needs no per-partition scale.

Host: input prep, percentile threshold via top-K selection over logits
refined in fp64 inside a window around the cut (reference fp32 scores
are ~1e-6 apart there; the device's float32r error is ~1e-4, so the
window is refined exactly to make the kept set match the reference's),
min-edges repair with fp64 refinement of numerically tight groups,
scatter to dense [B, N, N]. Unstructured inputs fall back to a full
host computation.
"""
import numpy as np

import concourse.bacc as bacc
import concourse.mybir as mybir
from concourse.tile import TileContext
from concourse.bass_utils import run_bass_kernel_spmd

B, N, DEG, D, ED = 4, 4096, 16, 128, 256
E = N * DEG
EPC = E // 2            # edges per core (two cores per batch)
SRC_PC = N // 2         # sources per core
NBLK = SRC_PC // 512    # 512-source blocks per core
NT = NBLK * DEG         # 512-edge tiles per core (64)
F32 = mybir.dt.float32
F32R = mybir.dt.float32r

_CACHE = {}


def _build(c_off):
    """c_off: tuple of DEG compile-time target offsets (tgt = src + roll + c_off[k])."""
    nc = bacc.Bacc("TRN2", target_bir_lowering=False, debug=False, num_devices=8)
    nfs = nc.dram_tensor("nfs", [D, SRC_PC], F32, kind="ExternalInput")
    nft = nc.dram_tensor("nft", [D, N + 512], F32, kind="ExternalInput")
    w_a1a = nc.dram_tensor("w_a1a", [D, D], F32, kind="ExternalInput")
    w_a1b = nc.dram_tensor("w_a1b", [D, D], F32, kind="ExternalInput")
    w_a2 = nc.dram_tensor("w_a2", [D, D], F32, kind="ExternalInput")
    w_1s = nc.dram_tensor("w_1s", [D, ED], F32, kind="ExternalInput")
    w_1t = nc.dram_tensor("w_1t", [D, ED], F32, kind="ExternalInput")
    w_1d = nc.dram_tensor("w_1d", [D, ED], F32, kind="ExternalInput")
    w_2 = nc.dram_tensor("w_2", [ED, D], F32, kind="ExternalInput")
    w_3 = nc.dram_tensor("w_3", [D, 1], F32, kind="ExternalInput")
    # col0 ba1, col1 ba2, col2 s1a, col3 s1b, col4 t1a, col5 t1b, col6 s2, col7 t2
    vb = nc.dram_tensor("vb", [D, 8], F32, kind="ExternalInput")
    lg = nc.dram_tensor("lg", [NT, 512], F32, kind="ExternalOutput")

    def r(ap):
        return ap.bitcast(F32R)

    with TileContext(nc) as tc:
        with tc.tile_pool(name="const", bufs=1) as cp, \
             tc.tile_pool(name="sb", bufs=3) as sb, \
             tc.tile_pool(name="ps_a", bufs=2, space="PSUM") as ps_a, \
             tc.tile_pool(name="ps_h", bufs=2, space="PSUM") as ps_h, \
             tc.tile_pool(name="ps_i", bufs=2, space="PSUM") as ps_i, \
             tc.tile_pool(name="ps_s", bufs=2, space="PSUM") as ps_s:
            nfs_t = cp.tile([D, SRC_PC], F32)
            nft_t = cp.tile([D, N + 512], F32)
            wa1a = cp.tile([D, D], F32)
            wa1b = cp.tile([D, D], F32)
            wa2 = cp.tile([D, D], F32)
            w1s = cp.tile([D, ED], F32)
            w1t = cp.tile([D, ED], F32)
            w1d = cp.tile([D, ED], F32)
            w2a = cp.tile([D, D], F32)
            w2b = cp.tile([D, D], F32)
            w3t = cp.tile([D, 1], F32)
            vbt = cp.tile([D, 8], F32)
            ones_k = cp.tile([D, 1], F32)    # lhsT for column sums
            ones_b = cp.tile([1, D], F32)    # lhsT for partition broadcast
            nc.sync.dma_start(out=nfs_t[:], in_=nfs[:])
            nc.sync.dma_start(out=nft_t[:], in_=nft[:])
            nc.sync.dma_start(out=wa1a[:], in_=w_a1a[:])
            nc.sync.dma_start(out=wa1b[:], in_=w_a1b[:])
            nc.sync.dma_start(out=wa2[:], in_=w_a2[:])
            nc.sync.dma_start(out=w1s[:], in_=w_1s[:])
            nc.sync.dma_start(out=w1t[:], in_=w_1t[:])
            nc.sync.dma_start(out=w1d[:], in_=w_1d[:])
            nc.sync.dma_start(out=w2a[:], in_=w_2[0:D, :])
            nc.sync.dma_start(out=w2b[:], in_=w_2[D:ED, :])
            nc.sync.dma_start(out=w3t[:], in_=w_3[:])
            nc.sync.dma_start(out=vbt[:], in_=vb[:])
            nc.vector.memset(ones_k[:], 1.0)
            nc.vector.memset(ones_b[:], 1.0)

            def stage_a(t):
                blk, k = t // DEG, t % DEG
                s0 = blk * 512
                toff = (s0 + c_off[k]) % N
                sfT = nfs_t[:, s0:s0 + 512]
                tfT = nft_t[:, toff:toff + 512]
                # attention layer 1: relu(Wa1a.T sf + Wa1b.T tf + ba1)
                p_a1 = ps_a.tile([D, 512], F32, space="PSUM", tag="pa")
                nc.tensor.matmul(out=p_a1[:], lhsT=wa1a[:], rhs=sfT,
                                 start=True, stop=False)
                nc.tensor.matmul(out=p_a1[:], lhsT=wa1b[:], rhs=tfT,
                                 start=False, stop=True)
                a1 = sb.tile([D, 512], F32R, tag="a1")
                nc.vector.scalar_tensor_tensor(
                    out=a1[:], in0=p_a1[:], scalar=vbt[:, 0:1], in1=zeros[:],
                    op0=mybir.AluOpType.add, op1=mybir.AluOpType.max)
                # attention layer 2 + exp (no max-subtract; |x| < 1)
                p_a2 = ps_a.tile([D, 512], F32, space="PSUM", tag="pa")
                nc.tensor.matmul(out=p_a2[:], lhsT=wa2[:], rhs=a1[:],
                                 start=True, stop=True)
                ex = sb.tile([D, 512], F32R, tag="ex")
                nc.scalar.activation(out=ex[:], in_=p_a2[:],
                                     func=mybir.ActivationFunctionType.Exp,
                                     bias=vbt[:, 1:2])
                # softmax denominator: cross-partition sum, broadcast
                sum_bc = sb.tile([D, 512], F32, tag="sum_bc")
                nc.gpsimd.partition_all_reduce(sum_bc[:], ex[:], channels=D,
                                               reduce_op=bass_isa.ReduceOp.add)
                # fda = |sf - tf| * ex / sum
                dif = sb.tile([D, 512], F32, tag="dif")
                nc.gpsimd.tensor_sub(out=dif[:], in0=nfs_s[:, s0:s0 + 512],
                                     in1=nft_s[:, toff:toff + 512])
                uu = sb.tile([D, 512], F32, tag="uu")
                nc.gpsimd.tensor_mul(out=uu[:], in0=dif[:], in1=ex[:])
                tt = sb.tile([D, 512], F32, tag="tt")
                nc.vector.scalar_tensor_tensor(
                    out=tt[:], in0=uu[:], scalar=-1.0, in1=uu[:],
                    op0=mybir.AluOpType.mult, op1=mybir.AluOpType.max)
                rcp = sb.tile([D, 512], F32, tag="rcp")
                nc.vector.reciprocal(out=rcp[:], in_=sum_bc[:])
                fda = sb.tile([D, 512], F32R, tag="fda")
                with nc.allow_low_precision(reason="f32r is 4-byte"):
                    nc.gpsimd.tensor_mul(out=fda[:], in0=tt[:], in1=rcp[:])
                return sfT, tfT, fda

            def stage_b(t, sfT, tfT, fda):
                # h1 = relu(s1*(W1s.T sf + W1t.T tf + W1d.T fda) + t1), 2 banks
                h1 = sb.tile([D, 1024], F32R, tag="h1")
                for bank in range(2):
                    cs = bank * D
                    p_h = ps_h.tile([D, 512], F32, space="PSUM", tag="ph")
                    nc.tensor.matmul(out=p_h[:], lhsT=w1s[:, cs:cs + D],
                                     rhs=sfT, start=True, stop=False)
                    nc.tensor.matmul(out=p_h[:], lhsT=w1t[:, cs:cs + D],
                                     rhs=tfT, start=False, stop=False)
                    nc.tensor.matmul(out=p_h[:], lhsT=w1d[:, cs:cs + D],
                                     rhs=fda[:], start=False, stop=True)
                    nc.scalar.activation(out=h1[:, bank * 512:(bank + 1) * 512],
                                         in_=p_h[:],
                                         func=mybir.ActivationFunctionType.Relu,
                                         bias=vbt[:, 4 + bank:5 + bank],
                                         scale=vbt[:, 2 + bank:3 + bank])
                # h2 = relu(W2'.T h1 + t2/s2)  (s2 folded into W3 on host)
                p_h2 = ps_h.tile([D, 512], F32, space="PSUM", tag="ph")
                nc.tensor.matmul(out=p_h2[:], lhsT=w2a[:],
                                 rhs=h1[:, 0:512], start=True, stop=False)
                nc.tensor.matmul(out=p_h2[:], lhsT=w2b[:],
                                 rhs=h1[:, 512:1024], start=False, stop=True)
                h2 = sb.tile([D, 512], F32R, tag="h2")
                nc.vector.scalar_tensor_tensor(
                    out=h2[:], in0=p_h2[:], scalar=vbt[:, 7:8], in1=zeros[:],
                    op0=mybir.AluOpType.add, op1=mybir.AluOpType.max)
                # logits = W3'.T h2 -> [1, 512]
                p_l = ps_s.tile([1, 512], F32, space="PSUM", tag="ps")
                nc.tensor.matmul(out=p_l[:], lhsT=w3t[:], rhs=h2[:],
                                 start=True, stop=True)
                lrow = sb.tile([1, 512], F32, tag="lrow")
                nc.scalar.copy(out=lrow[:], in_=p_l[:])
                nc.sync.dma_start(out=lg[t:t + 1, :], in_=lrow[:])

            # 2-stage software pipeline: attention front of tile t runs
            # interleaved (in program order) with the MLP back of tile t-1.
            import os as _os2
            LEAD = int(_os2.environ.get('KLEAD', '2'))
            pend = []
            for t in range(NT + LEAD):
                if t < NT:
                    pend.append((t, stage_a(t)))
                if t >= LEAD:
                    bt, args = pend.pop(0)
                    stage_b(bt, *args)
    nc.compile()
    return nc


def _detect_structure(src_idx, tgt_idx):
    """If src is grouped (e // DEG) and tgt = (src + d[b, k]) % N with
    d[b, k] = roll_b + c_k (c_k shared across batches), return c_off.
    Else None."""
    e_idx = np.arange(E, dtype=np.int64)
    if not (src_idx == (e_idx // DEG)[None, :]).all():
        return None
    d = (tgt_idx.astype(np.int64) - src_idx.astype(np.int64)) % N  # [B, E]
    d = d.reshape(B, N, DEG)
    if not (d == d[:, :1, :]).all():
        return None
    d = d[:, 0, :]  # [B, DEG]
    c = (d - d[:, :1]) % N
    if not (c == c[:1]).all():
        return None
    return tuple(int(x) for x in c[0]), [int(x) for x in d[:, 0]]


def _sigmoid64(x):
    return 1.0 / (1.0 + np.exp(-x.astype(np.float64)))


class _Refiner:
    """Exact (fp64) recompute of per-edge logits, mirroring the reference."""

    def __init__(self, inputs, nfn64):
        self.nfn64 = nfn64
        self.src = np.asarray(inputs["src_idx"], np.int64)
        self.tgt = np.asarray(inputs["tgt_idx"], np.int64)
        self.Wa1 = np.asarray(inputs["Wa1"], np.float64)
        self.ba1 = np.asarray(inputs["ba1"], np.float64)
        self.Wa2 = np.asarray(inputs["Wa2"], np.float64)
        self.ba2 = np.asarray(inputs["ba2"], np.float64)
        self.W1 = np.asarray(inputs["W1"], np.float64)
        self.b1 = np.asarray(inputs["b1"], np.float64)
        self.W2 = np.asarray(inputs["W2"], np.float64)
        self.b2 = np.asarray(inputs["b2"], np.float64)
        self.W3 = np.asarray(inputs["W3"], np.float64)
        self.b3 = np.asarray(inputs["b3"], np.float64)
        g1 = np.asarray(inputs["g1"], np.float64); v1 = np.asarray(inputs["v1"], np.float64)
        m1 = np.asarray(inputs["m1"], np.float64); be1 = np.asarray(inputs["be1"], np.float64)
        g2 = np.asarray(inputs["g2"], np.float64); v2 = np.asarray(inputs["v2"], np.float64)
        m2 = np.asarray(inputs["m2"], np.float64); be2 = np.asarray(inputs["be2"], np.float64)
        self.s1 = g1 / np.sqrt(v1 + 1e-5); self.t1 = be1 - m1 * self.s1
        self.s2 = g2 / np.sqrt(v2 + 1e-5); self.t2 = be2 - m2 * self.s2

    def logits(self, b, eids):
        if len(eids) == 0:
            return np.zeros((0,), np.float64)
        sf = self.nfn64[b][self.src[b, eids]]
        tf = self.nfn64[b][self.tgt[b, eids]]
        fd = np.abs(sf - tf)
        raw = np.concatenate([sf, tf], -1)
        a = np.maximum(raw @ self.Wa1 + self.ba1, 0.0) @ self.Wa2 + self.ba2
        e_ = np.exp(a - a.max(-1, keepdims=True))
        att = e_ / e_.sum(-1, keepdims=True)
        ef = np.concatenate([sf, tf, fd * att], -1)
        h = np.maximum((ef @ self.W1 + self.b1) * self.s1 + self.t1, 0.0)
        h = np.maximum((h @ self.W2 + self.b2) * self.s2 + self.t2, 0.0)
        return (h @ self.W3 + self.b3)[:, 0]


# refinement windows (logit space); measured device fp32r logit error is
# <= ~1.3e-4, so 5e-4 gives ~4x margin
W_LOGIT = 5e-4
W_GROUP = 5e-4


def _host_post(logits, inputs, refiner):
    """Threshold + min-edges repair + scatter with fp64 refinement near
    all decision boundaries."""
    src_idx = np.asarray(inputs["src_idx"], np.int64)
    tgt_idx = np.asarray(inputs["tgt_idx"], np.int64)
    me = int(np.asarray(inputs["min_edges_per_node"]))
    thr_idx = min(E * 50 // 100, E - 1)
    out = np.zeros((B, N, N), np.float32)
    for b in range(B):
        lg = logits[b].astype(np.float64).copy()
        # window refinement around the percentile cut
        lsort = np.sort(lg)
        lthr0 = lsort[E - 1 - thr_idx]
        cand = np.where(np.abs(lg - lthr0) <= W_LOGIT)[0]
        lg[cand] = refiner.logits(b, cand)
        # kept set = top-K by refined logit (the reference has no fp32 score
        # ties at its boundary; rank selection avoids rounding-tie artifacts)
        K = thr_idx + 1
        order = np.argsort(-lg, kind="stable")
        above = np.zeros(E, np.bool_)
        above[order[:K]] = True
        s = _sigmoid64(np.float32(lg)).astype(np.float32)
        grp_s = s.reshape(N, DEG)
        grp_a = above.reshape(N, DEG)
        active = grp_a.sum(-1)
        need = np.where(active < me, np.minimum(me - active, DEG), 0)
        # refine groups whose repair boundary is numerically tight
        rep = np.where(need > 0)[0]
        if len(rep):
            gs = np.sort(grp_s[rep], axis=-1)[:, ::-1]
            nd = need[rep]
            lo = gs[np.arange(len(rep)), nd - 1]
            hi = gs[np.arange(len(rep)), np.minimum(nd, DEG - 1)]
            tight = rep[(lo - hi) < W_GROUP]
            if len(tight):
                eids = (tight[:, None] * DEG + np.arange(DEG)[None, :]).reshape(-1)
                lg[eids] = refiner.logits(b, eids)
                s2 = _sigmoid64(np.float32(lg[eids])).astype(np.float32)
                grp_s[tight] = s2.reshape(len(tight), DEG)
        rank = np.argsort(np.argsort(-grp_s, axis=-1, kind="stable"),
                          axis=-1, kind="stable")
        keep = grp_a | (rank < need[:, None])
        final = np.where(keep, grp_s, 0.0).reshape(E)
        out[b, src_idx[b], tgt_idx[b]] = final
    return out


def _host_logits(nfn32, inputs):
    """Fallback full-precision host path for unstructured inputs."""
    refiner_like = _Refiner(inputs, nfn32.astype(np.float64))
    logits = np.zeros((B, E), np.float32)
    allall = np.arange(E)
    for b in range(B):
        logits[b] = refiner_like.logits(b, allall).astype(np.float32)
    return logits


def kernel(**inputs):
    node_feat = np.asarray(inputs["node_feat"], np.float32)
    src_idx = np.asarray(inputs["src_idx"], np.int32)
    tgt_idx = np.asarray(inputs["tgt_idx"], np.int32)

    # l2-normalize node features (fp64 accumulate, fp32 values for device)
    nf64 = node_feat.astype(np.float64)
    nrm = np.maximum(np.linalg.norm(nf64, axis=-1, keepdims=True), 1e-12)
    nfn64 = nf64 / nrm
    nfn = nfn64.astype(np.float32)

    refiner = _Refiner(inputs, nfn64)

    det = _detect_structure(src_idx, tgt_idx)
    if det is None:
        logits = _host_logits(nfn, inputs)
        return _host_post(logits, inputs, refiner)
    c_off, roll_b = det

    g1 = np.asarray(inputs["g1"], np.float64); be1 = np.asarray(inputs["be1"], np.float64)
    m1 = np.asarray(inputs["m1"], np.float64); v1 = np.asarray(inputs["v1"], np.float64)
    g2 = np.asarray(inputs["g2"], np.float64); be2 = np.asarray(inputs["be2"], np.float64)
    m2 = np.asarray(inputs["m2"], np.float64); v2 = np.asarray(inputs["v2"], np.float64)
    b1 = np.asarray(inputs["b1"], np.float64); b2 = np.asarray(inputs["b2"], np.float64)
    b3 = np.asarray(inputs["b3"], np.float64)
    s1 = (g1 / np.sqrt(v1 + 1e-5)); t1 = (b1 - m1) * s1 + be1
    s2 = (g2 / np.sqrt(v2 + 1e-5)); t2 = (b2 - m2) * s2 + be2
    # fold b3 into t2? no: logits = h2 @ W3 + b3; b3 is zero-filled per spec
    # but handle nonzero b3 by adding on host below.

    vb = np.zeros((D, 8), np.float32)
    vb[:, 0] = np.asarray(inputs["ba1"], np.float32)
    vb[:, 1] = np.asarray(inputs["ba2"], np.float32)
    vb[:, 2] = s1[0:D].astype(np.float32); vb[:, 3] = s1[D:ED].astype(np.float32)
    vb[:, 4] = t1[0:D].astype(np.float32); vb[:, 5] = t1[D:ED].astype(np.float32)
    vb[:, 6] = s2.astype(np.float32); vb[:, 7] = t2.astype(np.float32)

    key = ("nc", c_off)
    if key not in _CACHE:
        _CACHE[key] = _build(c_off)
    nc = _CACHE[key]

    Wa1 = np.asarray(inputs["Wa1"], np.float32)
    W1 = np.asarray(inputs["W1"], np.float32)
    w_maps_const = {
        "w_a1a": np.ascontiguousarray(Wa1[0:D]),
        "w_a1b": np.ascontiguousarray(Wa1[D:2 * D]),
        "w_a2": np.asarray(inputs["Wa2"], np.float32),
        "w_1s": np.ascontiguousarray(W1[0:D]),
        "w_1t": np.ascontiguousarray(W1[D:2 * D]),
        "w_1d": np.ascontiguousarray(W1[2 * D:3 * D]),
        "w_2": np.asarray(inputs["W2"], np.float32),
        "w_3": np.asarray(inputs["W3"], np.float32),
        "vb": vb,
    }

    in_maps = []
    for c in range(8):
        b, h = c // 2, c % 2
        nfT = nfn[b].T  # [D, N]
        roll = (roll_b[b] + h * SRC_PC) % N
        nft_roll = np.roll(nfT, -roll, axis=1)
        nft_ext = np.concatenate([nft_roll, nft_roll[:, :512]], axis=1)
        m = {
            "nfs": np.ascontiguousarray(nfT[:, h * SRC_PC:(h + 1) * SRC_PC]),
            "nft": np.ascontiguousarray(nft_ext),
        }
        m.update(w_maps_const)
        in_maps.append(m)

    res = run_bass_kernel_spmd(nc, in_maps, list(range(8)))
    logits = np.zeros((B, E), np.float32)
    for c in range(8):
        b, h = c // 2, c % 2
        # lg [NT, 512]: tile t = blk*DEG + k, col j = local src offset
        arr = res.results[c]["lg"].reshape(NBLK, DEG, 512)
        half = np.transpose(arr, (0, 2, 1)).reshape(SRC_PC * DEG)
        logits[b, h * SRC_PC * DEG:(h + 1) * SRC_PC * DEG] = half
    if b3[0] != 0.0:
        logits = (logits.astype(np.float64) + b3[0]).astype(np.float32)

    return _host_post(logits, inputs, refiner)
